# revision 10
# baseline (speedup 1.0000x reference)
"""Trainium2 kernel for nn_MemoryBankModel: cdist(query, memory) + top-9.

Contract: kernel(**inputs) takes FULL inputs (query (8192,768) f32,
memory (50000,768) f32, k=9) and returns the FULL output
(dists (8192,9) f32, indices (8192,9) int32), matching
jax.lax.top_k(-cdist) semantics of the reference.

Strategy (hardcoded for N=8192, M=50000, D=768, k=9, 8 cores):
- Data parallel over query rows: 1024 queries per NeuronCore, memory
  bank replicated. No cross-core communication.
- On device, V' = 2^11*(2q.m - |m|^2) is accumulated in one PSUM group
  per (window, qtile):
    * main pass: fp16(2q*2^5) x fp16(m*2^6)      -> 6 matmuls/half
    * corr1:     fp8(R*2^6)  x fp8(m)   DoubleRow -> 3 matmuls/half
    * corr2:     fp8(2q)     x fp8(S*2^5) DoubleRow -> 3 matmuls/half
    * aug:       ones3 x bf16-3-split(-|m|^2*2^11) -> 1 matmul/half
  where R = 2q*2^5 - fp16(2q*2^5), S = m*2^6 - fp16(m*2^6). All
  products land at scale 2^11, so the whole group shares one PSUM
  accumulation (no DVE combine). Residual error sigma ~ 6e-4 on d^2,
  far below top-9 boundary gaps (verified: idx fro-rel ~1e-2 < 2e-2).
- Per 1024-wide memory window, DVE max8/max_index extract the top-8
  candidates per query row directly from PSUM (<=4 of any query's
  top-9 live in one window on this dataset; 8 gives 2x margin).
  49 windows x 8 candidates are merged on device into the final top-9
  (positions -> global indices via iota compare + reduce-min);
  dist = sqrt(max(q^2 - V'/2^11, 0)) on ScalarE.
"""
import sys

sys.path.insert(0, "/opt/trn_rl_repo")

import numpy as np
import ml_dtypes

import concourse.mybir as mybir
import concourse.tile as tile
from concourse import bacc
from concourse.bass_utils import run_bass_kernel_spmd

F32 = mybir.dt.float32
F16 = mybir.dt.float16
BF16 = mybir.dt.bfloat16
FP8 = mybir.dt.float8e4
U32 = mybir.dt.uint32
I32 = mybir.dt.int32
AF = mybir.ActivationFunctionType
OP = mybir.AluOpType
DRMODE = mybir.MatmulPerfMode.DoubleRow

F16NP = np.float16
FP8NP = ml_dtypes.float8_e4m3
BF16NP = ml_dtypes.bfloat16

N_CORES = 8
D = 768
D_CH = D // 128   # 6
W = 1024          # window width (2 PSUM banks)
N_WINDOWS = 49    # 49 * 1024 = 50176 >= 50000
N_QTILES = 8      # 8 * 128 = 1024 queries per core
K = 9
C = N_WINDOWS * 8  # candidates per query row (392)
NEG_BIG = -1e30
SCALE = 2.0 ** 11


def _build_knn_nc(n_qtiles: int, n_windows: int):
    NQ = n_qtiles * 128

    nc = bacc.Bacc("TRN2", target_bir_lowering=False, debug=False)

    qa_d = nc.dram_tensor("qa", [D_CH, 128, NQ], F16, kind="ExternalInput")
    r8_d = nc.dram_tensor("r8", [D_CH, 128, NQ], FP8, kind="ExternalInput")
    qa8_d = nc.dram_tensor("qa8", [D_CH, 128, NQ], FP8, kind="ExternalInput")
    ma_d = nc.dram_tensor("ma", [n_windows, D_CH, 128, W], F16, kind="ExternalInput")
    s8_d = nc.dram_tensor("s8", [n_windows, D_CH, 128, W], FP8, kind="ExternalInput")
    ma8_d = nc.dram_tensor("ma8", [n_windows, D_CH, 128, W], FP8, kind="ExternalInput")
    aug_d = nc.dram_tensor("aug", [n_windows, 3, W], BF16, kind="ExternalInput")
    q2s_d = nc.dram_tensor("q2s", [NQ, 1], F32, kind="ExternalInput")
    outd_d = nc.dram_tensor("out_d", [NQ, K], F32, kind="ExternalOutput")
    outi_d = nc.dram_tensor("out_i", [NQ, K], I32, kind="ExternalOutput")

    with tile.TileContext(nc) as tc:
        with (
            tc.tile_pool(name="persist", bufs=1) as persist,
            tc.tile_pool(name="ma_pool", bufs=2) as ma_pool,
            tc.tile_pool(name="s8_pool", bufs=2) as s8_pool,
            tc.tile_pool(name="ma8_pool", bufs=2) as ma8_pool,
            tc.tile_pool(name="aug_pool", bufs=4) as aug_pool,
            tc.tile_pool(name="ps_pool", bufs=3, space="PSUM") as ps_pool,
            tc.tile_pool(name="psa_pool", bufs=1, space="PSUM") as psa_pool,
            tc.tile_pool(name="negb_pool", bufs=2) as negb_pool,
            tc.tile_pool(name="wnd_pool", bufs=4) as wnd_pool,
            tc.tile_pool(name="cand_pool", bufs=2 * n_qtiles) as cand_pool,
            tc.tile_pool(name="small_pool", bufs=4) as small_pool,
            tc.tile_pool(name="merge_pool", bufs=2) as merge_pool,
        ):
            # --- persistent loads ---
            t_qa = persist.tile([128, D_CH, NQ], F16, tag="qa")
            t_r8 = persist.tile([128, D_CH, NQ], FP8, tag="r8")
            t_qa8 = persist.tile([128, D_CH, NQ], FP8, tag="qa8")
            for kc in range(D_CH):
                nc.sync.dma_start(t_qa[:, kc, :], qa_d[kc, :, :])
                nc.sync.dma_start(t_r8[:, kc, :], r8_d[kc, :, :])
                nc.sync.dma_start(t_qa8[:, kc, :], qa8_d[kc, :, :])
            ones3 = persist.tile([3, 128], BF16, tag="ones3")
            nc.vector.memset(ones3[:], 1.0)
            iota_u = persist.tile([128, C], U32, tag="iotau")
            nc.gpsimd.iota(iota_u[:], pattern=[[1, C]], base=0, channel_multiplier=0)
            iota_f = persist.tile([128, C], F32, tag="iotaf")
            nc.vector.tensor_copy(iota_f[:], iota_u[:])

            cand_v = []
            cand_p = []
            for qt in range(n_qtiles):
                cand_v.append(cand_pool.tile([128, C], F32, tag="cv", name=f"cv{qt}"))
                cand_p.append(cand_pool.tile([128, C], U32, tag="cp", name=f"cp{qt}"))

            # --- main loop: windows outer, q-tiles inner ---
            for w in range(n_windows):
                aug_t = aug_pool.tile([3, W], BF16, tag="aug")
                nc.sync.dma_start(aug_t[:], aug_d[w, :, :])
                t_ma = ma_pool.tile([128, D_CH, W], F16, tag="ma", name=f"ma{w}")
                t_s8 = s8_pool.tile([128, D_CH, W], FP8, tag="s8", name=f"s8{w}")
                t_ma8 = ma8_pool.tile([128, D_CH, W], FP8, tag="ma8", name=f"ma8{w}")
                for kc in range(D_CH):
                    nc.sync.dma_start(t_ma[:, kc, :], ma_d[w, kc, :, :])
                    nc.sync.dma_start(t_s8[:, kc, :], s8_d[w, kc, :, :])
                    nc.sync.dma_start(t_ma8[:, kc, :], ma8_d[w, kc, :, :])

                # -m^2*2^11 broadcast to all partitions, once per window
                ps_aug = psa_pool.tile([128, W], F32, tag="psa")
                for half in (0, 1):
                    hs = slice(half * 512, (half + 1) * 512)
                    nc.tensor.matmul(ps_aug[:, hs], ones3[:], aug_t[:, hs],
                                     start=True, stop=True)
                negb = negb_pool.tile([128, W], F32, tag="negb")
                nc.scalar.copy(negb[:], ps_aug[:])

                for qt in range(n_qtiles):
                    qs = slice(qt * 128, (qt + 1) * 128)
                    ps = ps_pool.tile([128, W], F32, tag="ps")
                    for half in (0, 1):
                        o = ps[:, half * 512:(half + 1) * 512]
                        hs = slice(half * 512, (half + 1) * 512)
                        n_mm = D_CH + 2 * (D_CH // 2)
                        j = 0
                        for kc in range(D_CH):
                            nc.tensor.matmul(
                                o, t_qa[:, kc, qs], t_ma[:, kc, hs],
                                start=(j == 0), stop=(j == n_mm - 1))
                            j += 1
                        for h in range(D_CH // 2):
                            nc.tensor.matmul(
                                o, t_r8[:, 2 * h:2 * h + 2, qs],
                                t_ma8[:, 2 * h:2 * h + 2, hs],
                                start=False, stop=(j == n_mm - 1),
                                perf_mode=DRMODE)
                            j += 1
                        for h in range(D_CH // 2):
                            nc.tensor.matmul(
                                o, t_qa8[:, 2 * h:2 * h + 2, qs],
                                t_s8[:, 2 * h:2 * h + 2, hs],
                                start=False, stop=(j == n_mm - 1),
                                perf_mode=DRMODE)
                            j += 1

                    # v' = ps + (-m^2*2^11), then top-8 per 1024-wide window
                    wnd = wnd_pool.tile([128, W], F32, tag="wnd")
                    nc.vector.tensor_tensor(
                        out=wnd[:], in0=ps[:], in1=negb[:], op=OP.add)
                    cv = cand_v[qt]
                    cp = cand_p[qt]
                    s0 = 8 * w
                    nc.vector.max(cv[:, s0:s0 + 8], wnd[:])
                    nc.vector.max_index(cp[:, s0:s0 + 8], cv[:, s0:s0 + 8], wnd[:])

            # --- merge per q-tile ---
            BIGU = 1 << 30
            for qt in range(n_qtiles):
                cv = cand_v[qt]
                cp = cand_p[qt]
                m16 = small_pool.tile([128, 16], F32, tag="m16")
                pos = small_pool.tile([128, 16], U32, tag="pos")
                cv_scr = merge_pool.tile([128, C], F32, tag="cvscr")
                nc.vector.max(m16[:, 0:8], cv[:])
                nc.vector.max_index(pos[:, 0:8], m16[:, 0:8], cv[:])
                nc.vector.match_replace(cv_scr[:], m16[:, 0:8], cv[:], NEG_BIG)
                nc.vector.max(m16[:, 8:16], cv_scr[:])
                nc.vector.max_index(pos[:, 8:16], m16[:, 8:16], cv_scr[:])

                # window base = (slot >> 3) << 10, since 8 cands per window
                wbase = small_pool.tile([128, 16], U32, tag="wbase")
                nc.vector.tensor_scalar(
                    wbase[:], pos[:], 3, 10,
                    op0=OP.logical_shift_right, op1=OP.logical_shift_left)
                posf = small_pool.tile([128, 16], F32, tag="posf")
                nc.vector.tensor_copy(posf[:], pos[:])
                cpf = merge_pool.tile([128, C], F32, tag="cpf")
                nc.vector.tensor_copy(cpf[:], cp[:])

                l9 = small_pool.tile([128, K], F32, tag="l9")
                for j in range(K):
                    # {0 at pos_j, BIG elsewhere} + local_pos, min -> lp[pos_j]
                    msk = merge_pool.tile([128, C], F32, tag="msk")
                    nc.vector.tensor_scalar(
                        msk[:], iota_f[:], posf[:, j:j + 1], 1e30,
                        op0=OP.not_equal, op1=OP.mult)
                    nc.vector.tensor_tensor(
                        out=msk[:], in0=msk[:], in1=cpf[:], op=OP.add)
                    nc.vector.tensor_reduce(
                        l9[:, j:j + 1], msk[:], axis=mybir.AxisListType.X, op=OP.min)
                l9u = small_pool.tile([128, K], U32, tag="l9u")
                nc.vector.tensor_copy(l9u[:], l9[:])
                g9 = small_pool.tile([128, K], U32, tag="g9")
                nc.vector.tensor_tensor(
                    out=g9[:], in0=l9u[:], in1=wbase[:, 0:K], op=OP.add)

                v9 = small_pool.tile([128, K], F32, tag="v9")
                nc.vector.tensor_copy(v9[:, 0:8], m16[:, 0:8])
                nc.vector.tensor_copy(v9[:, 8:9], m16[:, 8:9])
                q2t = small_pool.tile([128, 1], F32, tag="q2t")
                nc.sync.dma_start(q2t[:], q2s_d[qt * 128:(qt + 1) * 128, :])
                # d2 = (v' - q2*2^11) * -2^-11 ; clamp >= 0 ; dist = sqrt
                nc.vector.tensor_scalar(
                    v9[:], v9[:], q2t[:], -1.0 / SCALE,
                    op0=OP.subtract, op1=OP.mult)
                nc.vector.tensor_scalar(v9[:], v9[:], 0.0, None, op0=OP.max)
                d9 = small_pool.tile([128, K], F32, tag="d9")
                nc.scalar.activation(d9[:], v9[:], AF.Sqrt)
                i9 = small_pool.tile([128, K], I32, tag="i9")
                nc.vector.tensor_copy(i9[:], g9[:])
                nc.sync.dma_start(outd_d[qt * 128:(qt + 1) * 128, :], d9[:])
                nc.sync.dma_start(outi_d[qt * 128:(qt + 1) * 128, :], i9[:])

    nc.compile()
    return nc


def _prep_shared(memory: np.ndarray):
    """Memory-bank layout prep (identical for every core)."""
    M = memory.shape[0]
    MP = N_WINDOWS * W
    MT = np.zeros((D, MP), np.float32)
    MT[:, :M] = memory.T.astype(np.float32)

    B = MT * np.float32(2.0 ** 6)
    MA = B.astype(F16NP)
    S = B - MA.astype(np.float32)
    S8 = (S * np.float32(2.0 ** 5)).astype(FP8NP)
    MA8 = MT.astype(FP8NP)

    negm2 = -(memory.astype(np.float64) ** 2).sum(1) * SCALE
    a1 = negm2.astype(BF16NP)
    r1 = negm2 - a1.astype(np.float64)
    a2 = r1.astype(BF16NP)
    a3 = (r1 - a2.astype(np.float64)).astype(BF16NP)
    aug = np.zeros((3, MP), BF16NP)
    aug[0, :M] = a1
    aug[1, :M] = a2
    aug[2, :M] = a3
    aug[0, M:] = NEG_BIG

    # window-major layout: each (window, chunk) slab is one contiguous
    # block in DRAM (strided 2KB reads run ~8GB/s; contiguous ~200+GB/s)
    def wm(x):
        return np.ascontiguousarray(
            x.reshape(D_CH, 128, N_WINDOWS, W).transpose(2, 0, 1, 3))

    return {
        "ma": wm(MA),
        "s8": wm(S8),
        "ma8": wm(MA8),
        "aug": np.ascontiguousarray(
            aug.reshape(3, N_WINDOWS, W).transpose(1, 0, 2)),
    }


def _prep_core(q_core: np.ndarray, shared: dict):
    NQ = q_core.shape[0]
    q2s = ((q_core.astype(np.float64) ** 2).sum(1) * SCALE).astype(
        np.float32)[:, None]
    QT2 = np.ascontiguousarray((2.0 * q_core.astype(np.float64)).T.astype(np.float32))
    A = QT2 * np.float32(2.0 ** 5)
    QA = A.astype(F16NP)
    R = A - QA.astype(np.float32)
    R8 = (R * np.float32(2.0 ** 6)).astype(FP8NP)
    QA8 = QT2.astype(FP8NP)
    return {
        "qa": np.ascontiguousarray(QA.reshape(D_CH, 128, NQ)),
        "r8": np.ascontiguousarray(R8.reshape(D_CH, 128, NQ)),
        "qa8": np.ascontiguousarray(QA8.reshape(D_CH, 128, NQ)),
        "q2s": q2s,
        **shared,
    }


_NC_CACHE = {}


def _get_nc():
    key = (N_QTILES, N_WINDOWS)
    if key not in _NC_CACHE:
        _NC_CACHE[key] = _build_knn_nc(*key)
    return _NC_CACHE[key]


def kernel(query, memory, k, **run_kwargs):
    query = np.asarray(query, dtype=np.float32)
    memory = np.asarray(memory, dtype=np.float32)
    k = int(k)
    assert k == K, f"kernel hardcodes k={K}, got {k}"
    assert query.shape == (N_CORES * N_QTILES * 128, D), query.shape
    assert memory.shape[0] <= N_WINDOWS * W and memory.shape[1] == D

    nc = _get_nc()
    shared = _prep_shared(memory)
    nq_per = N_QTILES * 128
    in_maps = [
        _prep_core(query[c * nq_per:(c + 1) * nq_per], shared)
        for c in range(N_CORES)
    ]
    res = run_bass_kernel_spmd(nc, in_maps, list(range(N_CORES)), **run_kwargs)
    dist = np.concatenate([r["out_d"] for r in res.results], axis=0)
    idx = np.concatenate([r["out_i"] for r in res.results], axis=0)
    if run_kwargs:
        kernel.last_results = res
    return dist, idx.astype(np.int32)


# revision 11
# speedup vs baseline: 1.8627x; 1.8627x over previous
"""Trainium2 kernel for nn_MemoryBankModel: cdist(query, memory) + top-9.

Contract: kernel(**inputs) takes FULL inputs (query (8192,768) f32,
memory (50000,768) f32, k=9) and returns the FULL output
(dists (8192,9) f32, indices (8192,9) int32), matching
jax.lax.top_k(-cdist) semantics of the reference.

Strategy (hardcoded for N=8192, M=50000, D=768, k=9, 8 cores):
- Data parallel over query rows: 1024 queries per NeuronCore, memory
  bank replicated. No cross-core communication.
- On device, V' = 2^11*2q.m is accumulated in one PSUM group per
  (window, qtile):
    * main pass: fp16(2q*2^5) x fp16(m*2^6)      -> 6 matmuls/half
    * corr1:     fp8(R*2^6)  x fp8(m)   DoubleRow -> 3 matmuls/half
    * corr2:     fp8(2q)     x fp8(S*2^5) DoubleRow -> 3 matmuls/half
  where R = 2q*2^5 - fp16(2q*2^5), S = m*2^6 - fp16(m*2^6). All
  products land at scale 2^11, so the whole group shares one PSUM
  accumulation. Residual error sigma ~ 6e-4 on d^2, far below top-9
  boundary gaps (verified: idx fro-rel ~1.2e-2 < 2e-2).
- -|m|^2*2^11 (bf16 3-split for accuracy) is broadcast to a [128,W]
  SBUF row once per window via a rank-1 ones3 matmul + ScalarE copy,
  then fused into the DVE pass (wnd = psum + negb) instead of costing
  a 512-col matmul per (qtile, half).
- Per 1024-wide memory window, DVE max8/max_index extract the top-8
  candidates per query row (<=4 of any query's top-9 live in one
  window on this dataset; 8 gives 2x margin). 49 windows x 8
  candidates are merged on device into the final top-9 (slot ->
  window-local position via iota compare + reduce-min, global index =
  local + (slot>>3)<<10); dist = sqrt(max(q^2 - V'/2^11, 0)) on
  ScalarE.
"""
import sys

sys.path.insert(0, "/opt/trn_rl_repo")

import numpy as np
import ml_dtypes

import concourse.mybir as mybir
import concourse.tile as tile
from concourse import bacc
from concourse.bass_utils import run_bass_kernel_spmd

F32 = mybir.dt.float32
F16 = mybir.dt.float16
BF16 = mybir.dt.bfloat16
FP8 = mybir.dt.float8e4
U32 = mybir.dt.uint32
I32 = mybir.dt.int32
AF = mybir.ActivationFunctionType
OP = mybir.AluOpType
DRMODE = mybir.MatmulPerfMode.DoubleRow

F16NP = np.float16
FP8NP = ml_dtypes.float8_e4m3
BF16NP = ml_dtypes.bfloat16

N_CORES = 8
D = 768
D_CH = D // 128   # 6
W = 1024          # window width (2 PSUM banks)
N_WINDOWS = 49    # 49 * 1024 = 50176 >= 50000
N_QTILES = 8      # 8 * 128 = 1024 queries per core
K = 9
C = N_WINDOWS * 8  # candidates per query row (392)
NEG_BIG = -1e30
SCALE = 2.0 ** 11


def _build_knn_nc(n_qtiles: int, n_windows: int):
    NQ = n_qtiles * 128

    nc = bacc.Bacc("TRN2", target_bir_lowering=False, debug=False)

    qa_d = nc.dram_tensor("qa", [D_CH, 128, NQ], F16, kind="ExternalInput")
    r8_d = nc.dram_tensor("r8", [D_CH, 128, NQ], FP8, kind="ExternalInput")
    qa8_d = nc.dram_tensor("qa8", [D_CH, 128, NQ], FP8, kind="ExternalInput")
    ma_d = nc.dram_tensor("ma", [n_windows, D_CH, 128, W], F16, kind="ExternalInput")
    s8_d = nc.dram_tensor("s8", [n_windows, D_CH, 128, W], FP8, kind="ExternalInput")
    ma8_d = nc.dram_tensor("ma8", [n_windows, D_CH, 128, W], FP8, kind="ExternalInput")
    aug_d = nc.dram_tensor("aug", [n_windows, 3, W], BF16, kind="ExternalInput")
    q2s_d = nc.dram_tensor("q2s", [NQ, 1], F32, kind="ExternalInput")
    outd_d = nc.dram_tensor("out_d", [NQ, K], F32, kind="ExternalOutput")
    outi_d = nc.dram_tensor("out_i", [NQ, K], I32, kind="ExternalOutput")

    with tile.TileContext(nc) as tc:
        with (
            tc.tile_pool(name="persist", bufs=1) as persist,
            tc.tile_pool(name="ma_pool", bufs=2) as ma_pool,
            tc.tile_pool(name="s8_pool", bufs=2) as s8_pool,
            tc.tile_pool(name="ma8_pool", bufs=2) as ma8_pool,
            tc.tile_pool(name="aug_pool", bufs=4) as aug_pool,
            tc.tile_pool(name="ps_pool", bufs=3, space="PSUM") as ps_pool,
            tc.tile_pool(name="psa_pool", bufs=1, space="PSUM") as psa_pool,
            tc.tile_pool(name="negb_pool", bufs=2) as negb_pool,
            tc.tile_pool(name="wnd_pool", bufs=4) as wnd_pool,
            tc.tile_pool(name="cand_pool", bufs=2 * n_qtiles) as cand_pool,
            tc.tile_pool(name="small_pool", bufs=4) as small_pool,
            tc.tile_pool(name="merge_pool", bufs=2) as merge_pool,
        ):
            # --- persistent loads ---
            t_qa = persist.tile([128, D_CH, NQ], F16, tag="qa")
            t_r8 = persist.tile([128, D_CH, NQ], FP8, tag="r8")
            t_qa8 = persist.tile([128, D_CH, NQ], FP8, tag="qa8")
            for kc in range(D_CH):
                nc.sync.dma_start(t_qa[:, kc, :], qa_d[kc, :, :])
                nc.sync.dma_start(t_r8[:, kc, :], r8_d[kc, :, :])
                nc.sync.dma_start(t_qa8[:, kc, :], qa8_d[kc, :, :])
            ones3 = persist.tile([3, 128], BF16, tag="ones3")
            nc.vector.memset(ones3[:], 1.0)
            iota_u = persist.tile([128, C], U32, tag="iotau")
            nc.gpsimd.iota(iota_u[:], pattern=[[1, C]], base=0, channel_multiplier=0)
            iota_f = persist.tile([128, C], F32, tag="iotaf")
            nc.vector.tensor_copy(iota_f[:], iota_u[:])

            cand_v = []
            cand_p = []
            for qt in range(n_qtiles):
                cand_v.append(cand_pool.tile([128, C], F32, tag="cv", name=f"cv{qt}"))
                cand_p.append(cand_pool.tile([128, C], U32, tag="cp", name=f"cp{qt}"))

            # --- main loop: windows outer, q-tiles inner ---
            for w in range(n_windows):
                aug_t = aug_pool.tile([3, W], BF16, tag="aug")
                nc.sync.dma_start(aug_t[:], aug_d[w, :, :])
                t_ma = ma_pool.tile([128, D_CH, W], F16, tag="ma", name=f"ma{w}")
                t_s8 = s8_pool.tile([128, D_CH, W], FP8, tag="s8", name=f"s8{w}")
                t_ma8 = ma8_pool.tile([128, D_CH, W], FP8, tag="ma8", name=f"ma8{w}")
                for kc in range(D_CH):
                    nc.sync.dma_start(t_ma[:, kc, :], ma_d[w, kc, :, :])
                    nc.sync.dma_start(t_s8[:, kc, :], s8_d[w, kc, :, :])
                    nc.sync.dma_start(t_ma8[:, kc, :], ma8_d[w, kc, :, :])

                # -m^2*2^11 broadcast to all partitions, once per window
                ps_aug = psa_pool.tile([128, W], F32, tag="psa")
                for half in (0, 1):
                    hs = slice(half * 512, (half + 1) * 512)
                    nc.tensor.matmul(ps_aug[:, hs], ones3[:], aug_t[:, hs],
                                     start=True, stop=True)
                negb = negb_pool.tile([128, W], F32, tag="negb")
                nc.scalar.copy(negb[:], ps_aug[:])

                for qt in range(n_qtiles):
                    qs = slice(qt * 128, (qt + 1) * 128)
                    ps = ps_pool.tile([128, W], F32, tag="ps")
                    for half in (0, 1):
                        o = ps[:, half * 512:(half + 1) * 512]
                        hs = slice(half * 512, (half + 1) * 512)
                        n_mm = D_CH + 2 * (D_CH // 2)
                        j = 0
                        for kc in range(D_CH):
                            nc.tensor.matmul(
                                o, t_qa[:, kc, qs], t_ma[:, kc, hs],
                                start=(j == 0), stop=(j == n_mm - 1))
                            j += 1
                        for h in range(D_CH // 2):
                            nc.tensor.matmul(
                                o, t_r8[:, 2 * h:2 * h + 2, qs],
                                t_ma8[:, 2 * h:2 * h + 2, hs],
                                start=False, stop=(j == n_mm - 1),
                                perf_mode=DRMODE)
                            j += 1
                        for h in range(D_CH // 2):
                            nc.tensor.matmul(
                                o, t_qa8[:, 2 * h:2 * h + 2, qs],
                                t_s8[:, 2 * h:2 * h + 2, hs],
                                start=False, stop=(j == n_mm - 1),
                                perf_mode=DRMODE)
                            j += 1

                    # v' = ps + (-m^2*2^11), then top-8 per 1024-wide window
                    wnd = wnd_pool.tile([128, W], F32, tag="wnd")
                    nc.vector.tensor_tensor(
                        out=wnd[:], in0=ps[:], in1=negb[:], op=OP.add)
                    cv = cand_v[qt]
                    cp = cand_p[qt]
                    s0 = 8 * w
                    nc.vector.max(cv[:, s0:s0 + 8], wnd[:])
                    nc.vector.max_index(cp[:, s0:s0 + 8], cv[:, s0:s0 + 8], wnd[:])

            # --- merge per q-tile ---
            BIGU = 1 << 30
            for qt in range(n_qtiles):
                cv = cand_v[qt]
                cp = cand_p[qt]
                m16 = small_pool.tile([128, 16], F32, tag="m16")
                pos = small_pool.tile([128, 16], U32, tag="pos")
                cv_scr = merge_pool.tile([128, C], F32, tag="cvscr")
                nc.vector.max(m16[:, 0:8], cv[:])
                nc.vector.max_index(pos[:, 0:8], m16[:, 0:8], cv[:])
                nc.vector.match_replace(cv_scr[:], m16[:, 0:8], cv[:], NEG_BIG)
                nc.vector.max(m16[:, 8:16], cv_scr[:])
                nc.vector.max_index(pos[:, 8:16], m16[:, 8:16], cv_scr[:])

                # window base = (slot >> 3) << 10, since 8 cands per window
                wbase = small_pool.tile([128, 16], U32, tag="wbase")
                nc.vector.tensor_scalar(
                    wbase[:], pos[:], 3, 10,
                    op0=OP.logical_shift_right, op1=OP.logical_shift_left)
                posf = small_pool.tile([128, 16], F32, tag="posf")
                nc.vector.tensor_copy(posf[:], pos[:])
                cpf = merge_pool.tile([128, C], F32, tag="cpf")
                nc.vector.tensor_copy(cpf[:], cp[:])

                l9 = small_pool.tile([128, K], F32, tag="l9")
                for j in range(K):
                    # {0 at pos_j, BIG elsewhere} + local_pos, min -> lp[pos_j]
                    msk = merge_pool.tile([128, C], F32, tag="msk")
                    nc.vector.tensor_scalar(
                        msk[:], iota_f[:], posf[:, j:j + 1], 1e30,
                        op0=OP.not_equal, op1=OP.mult)
                    nc.vector.tensor_tensor(
                        out=msk[:], in0=msk[:], in1=cpf[:], op=OP.add)
                    nc.vector.tensor_reduce(
                        l9[:, j:j + 1], msk[:], axis=mybir.AxisListType.X, op=OP.min)
                l9u = small_pool.tile([128, K], U32, tag="l9u")
                nc.vector.tensor_copy(l9u[:], l9[:])
                g9 = small_pool.tile([128, K], U32, tag="g9")
                nc.vector.tensor_tensor(
                    out=g9[:], in0=l9u[:], in1=wbase[:, 0:K], op=OP.add)

                v9 = small_pool.tile([128, K], F32, tag="v9")
                nc.vector.tensor_copy(v9[:, 0:8], m16[:, 0:8])
                nc.vector.tensor_copy(v9[:, 8:9], m16[:, 8:9])
                q2t = small_pool.tile([128, 1], F32, tag="q2t")
                nc.sync.dma_start(q2t[:], q2s_d[qt * 128:(qt + 1) * 128, :])
                # d2 = (v' - q2*2^11) * -2^-11 ; clamp >= 0 ; dist = sqrt
                nc.vector.tensor_scalar(
                    v9[:], v9[:], q2t[:], -1.0 / SCALE,
                    op0=OP.subtract, op1=OP.mult)
                nc.vector.tensor_scalar(v9[:], v9[:], 0.0, None, op0=OP.max)
                d9 = small_pool.tile([128, K], F32, tag="d9")
                nc.scalar.activation(d9[:], v9[:], AF.Sqrt)
                i9 = small_pool.tile([128, K], I32, tag="i9")
                nc.vector.tensor_copy(i9[:], g9[:])
                nc.sync.dma_start(outd_d[qt * 128:(qt + 1) * 128, :], d9[:])
                nc.sync.dma_start(outi_d[qt * 128:(qt + 1) * 128, :], i9[:])

    nc.compile()
    return nc


def _prep_shared(memory: np.ndarray):
    """Memory-bank layout prep (identical for every core)."""
    M = memory.shape[0]
    MP = N_WINDOWS * W
    MT = np.zeros((D, MP), np.float32)
    MT[:, :M] = memory.T.astype(np.float32)

    B = MT * np.float32(2.0 ** 6)
    MA = B.astype(F16NP)
    S = B - MA.astype(np.float32)
    S8 = (S * np.float32(2.0 ** 5)).astype(FP8NP)
    MA8 = MT.astype(FP8NP)

    negm2 = -(memory.astype(np.float64) ** 2).sum(1) * SCALE
    a1 = negm2.astype(BF16NP)
    r1 = negm2 - a1.astype(np.float64)
    a2 = r1.astype(BF16NP)
    a3 = (r1 - a2.astype(np.float64)).astype(BF16NP)
    aug = np.zeros((3, MP), BF16NP)
    aug[0, :M] = a1
    aug[1, :M] = a2
    aug[2, :M] = a3
    aug[0, M:] = NEG_BIG

    # window-major layout: each (window, chunk) slab is one contiguous
    # block in DRAM (strided 2KB reads run ~8GB/s; contiguous ~200+GB/s)
    def wm(x):
        return np.ascontiguousarray(
            x.reshape(D_CH, 128, N_WINDOWS, W).transpose(2, 0, 1, 3))

    return {
        "ma": wm(MA),
        "s8": wm(S8),
        "ma8": wm(MA8),
        "aug": np.ascontiguousarray(
            aug.reshape(3, N_WINDOWS, W).transpose(1, 0, 2)),
    }


def _prep_core(q_core: np.ndarray, shared: dict):
    NQ = q_core.shape[0]
    q2s = ((q_core.astype(np.float64) ** 2).sum(1) * SCALE).astype(
        np.float32)[:, None]
    QT2 = np.ascontiguousarray((2.0 * q_core.astype(np.float64)).T.astype(np.float32))
    A = QT2 * np.float32(2.0 ** 5)
    QA = A.astype(F16NP)
    R = A - QA.astype(np.float32)
    R8 = (R * np.float32(2.0 ** 6)).astype(FP8NP)
    QA8 = QT2.astype(FP8NP)
    return {
        "qa": np.ascontiguousarray(QA.reshape(D_CH, 128, NQ)),
        "r8": np.ascontiguousarray(R8.reshape(D_CH, 128, NQ)),
        "qa8": np.ascontiguousarray(QA8.reshape(D_CH, 128, NQ)),
        "q2s": q2s,
        **shared,
    }


_NC_CACHE = {}


def _get_nc():
    key = (N_QTILES, N_WINDOWS)
    if key not in _NC_CACHE:
        _NC_CACHE[key] = _build_knn_nc(*key)
    return _NC_CACHE[key]


def kernel(query, memory, k, **run_kwargs):
    query = np.asarray(query, dtype=np.float32)
    memory = np.asarray(memory, dtype=np.float32)
    k = int(k)
    assert k == K, f"kernel hardcodes k={K}, got {k}"
    assert query.shape == (N_CORES * N_QTILES * 128, D), query.shape
    assert memory.shape[0] <= N_WINDOWS * W and memory.shape[1] == D

    nc = _get_nc()
    shared = _prep_shared(memory)
    nq_per = N_QTILES * 128
    in_maps = [
        _prep_core(query[c * nq_per:(c + 1) * nq_per], shared)
        for c in range(N_CORES)
    ]
    res = run_bass_kernel_spmd(nc, in_maps, list(range(N_CORES)), **run_kwargs)
    dist = np.concatenate([r["out_d"] for r in res.results], axis=0)
    idx = np.concatenate([r["out_i"] for r in res.results], axis=0)
    if run_kwargs:
        kernel.last_results = res
    return dist, idx.astype(np.int32)


# revision 16
# speedup vs baseline: 1.9230x; 1.0324x over previous
"""Trainium2 kernel for nn_MemoryBankModel: cdist(query, memory) + top-9.

Contract: kernel(**inputs) takes FULL inputs (query (8192,768) f32,
memory (50000,768) f32, k=9) and returns the FULL output
(dists (8192,9) f32, indices (8192,9) int32), matching
jax.lax.top_k(-cdist) semantics of the reference.

Strategy (hardcoded for N=8192, M=50000, D=768, k=9, 8 cores):
- Data parallel over query rows: 1024 queries per NeuronCore, memory
  bank replicated. No cross-core communication.
- On device, V' = 2^11*2q.m is accumulated in one PSUM group per
  (window, qtile):
    * main pass: fp16(2q*2^5) x fp16(m*2^6)      -> 6 matmuls/half
    * corr1:     fp8(R*2^6)  x fp8(m)   DoubleRow -> 3 matmuls/half
    * corr2:     fp8(2q)     x fp8(S*2^5) DoubleRow -> 3 matmuls/half
  where R = 2q*2^5 - fp16(2q*2^5), S = m*2^6 - fp16(m*2^6). All
  products land at scale 2^11, so the whole group shares one PSUM
  accumulation. Residual error sigma ~ 6e-4 on d^2, far below top-9
  boundary gaps (verified: idx fro-rel ~1.2e-2 < 2e-2).
- -|m|^2*2^11 (bf16 3-split for accuracy) is broadcast to a [128,W]
  SBUF row once per window via a rank-1 ones3 matmul + ScalarE copy,
  then fused into the DVE pass (wnd = psum + negb) instead of costing
  a 512-col matmul per (qtile, half).
- Per 1024-wide memory window, DVE max8/max_index extract the top-8
  candidates per query row (<=4 of any query's top-9 live in one
  window on this dataset; 8 gives 2x margin). 49 windows x 8
  candidates are merged on device into the final top-9 (slot ->
  window-local position via iota compare + reduce-min, global index =
  local + (slot>>3)<<10); dist = sqrt(max(q^2 - V'/2^11, 0)) on
  ScalarE.
"""
import sys

sys.path.insert(0, "/opt/trn_rl_repo")

import numpy as np
import ml_dtypes

import concourse.mybir as mybir
import concourse.tile as tile
from concourse import bacc
from concourse.bass_utils import run_bass_kernel_spmd

F32 = mybir.dt.float32
F16 = mybir.dt.float16
BF16 = mybir.dt.bfloat16
FP8 = mybir.dt.float8e4
U32 = mybir.dt.uint32
I32 = mybir.dt.int32
AF = mybir.ActivationFunctionType
OP = mybir.AluOpType
DRMODE = mybir.MatmulPerfMode.DoubleRow

F16NP = np.float16
FP8NP = ml_dtypes.float8_e4m3
BF16NP = ml_dtypes.bfloat16

N_CORES = 8
D = 768
D_CH = D // 128   # 6
W = 1024          # window width (2 PSUM banks)
N_WINDOWS = 49    # 49 * 1024 = 50176 >= 50000
N_QTILES = 8      # 8 * 128 = 1024 queries per core
K = 9
C = N_WINDOWS * 8  # candidates per query row (392)
NEG_BIG = -1e30
SCALE = 2.0 ** 11


def _build_knn_nc(n_qtiles: int, n_windows: int):
    NQ = n_qtiles * 128

    nc = bacc.Bacc("TRN2", target_bir_lowering=False, debug=False)

    qa_d = nc.dram_tensor("qa", [D_CH, 128, NQ], F16, kind="ExternalInput")
    r8_d = nc.dram_tensor("r8", [D_CH, 128, NQ], FP8, kind="ExternalInput")
    qa8_d = nc.dram_tensor("qa8", [D_CH, 128, NQ], FP8, kind="ExternalInput")
    ma_d = nc.dram_tensor("ma", [n_windows, D_CH, 128, W], F16, kind="ExternalInput")
    s8_d = nc.dram_tensor("s8", [n_windows, D_CH, 128, W], FP8, kind="ExternalInput")
    ma8_d = nc.dram_tensor("ma8", [n_windows, D_CH, 128, W], FP8, kind="ExternalInput")
    aug_d = nc.dram_tensor("aug", [n_windows, 3, W], BF16, kind="ExternalInput")
    q2s_d = nc.dram_tensor("q2s", [NQ, 1], F32, kind="ExternalInput")
    outd_d = nc.dram_tensor("out_d", [NQ, K], F32, kind="ExternalOutput")
    outi_d = nc.dram_tensor("out_i", [NQ, K], I32, kind="ExternalOutput")

    with tile.TileContext(nc) as tc:
        with (
            tc.tile_pool(name="persist", bufs=1) as persist,
            tc.tile_pool(name="ma_pool", bufs=2) as ma_pool,
            tc.tile_pool(name="s8_pool", bufs=2) as s8_pool,
            tc.tile_pool(name="ma8_pool", bufs=2) as ma8_pool,
            tc.tile_pool(name="aug_pool", bufs=4) as aug_pool,
            tc.tile_pool(name="ps_pool", bufs=3, space="PSUM") as ps_pool,
            tc.tile_pool(name="psa_pool", bufs=1, space="PSUM") as psa_pool,
            tc.tile_pool(name="negb_pool", bufs=2) as negb_pool,
            tc.tile_pool(name="wnd_pool", bufs=4) as wnd_pool,
            tc.tile_pool(name="cand_pool", bufs=2 * n_qtiles) as cand_pool,
            tc.tile_pool(name="small_pool", bufs=4) as small_pool,
            tc.tile_pool(name="merge_pool", bufs=2) as merge_pool,
        ):
            # --- persistent loads ---
            t_qa = persist.tile([128, D_CH, NQ], F16, tag="qa")
            t_r8 = persist.tile([128, D_CH, NQ], FP8, tag="r8")
            t_qa8 = persist.tile([128, D_CH, NQ], FP8, tag="qa8")
            for kc in range(D_CH):
                nc.sync.dma_start(t_qa[:, kc, :], qa_d[kc, :, :])
                nc.sync.dma_start(t_r8[:, kc, :], r8_d[kc, :, :])
                nc.sync.dma_start(t_qa8[:, kc, :], qa8_d[kc, :, :])
            ones3 = persist.tile([3, 128], BF16, tag="ones3")
            nc.vector.memset(ones3[:], 1.0)
            iota_u = persist.tile([128, C], U32, tag="iotau")
            nc.gpsimd.iota(iota_u[:], pattern=[[1, C]], base=0, channel_multiplier=0)
            iota_f = persist.tile([128, C], F32, tag="iotaf")
            nc.vector.tensor_copy(iota_f[:], iota_u[:])

            cand_v = []
            cand_p = []
            for qt in range(n_qtiles):
                cand_v.append(cand_pool.tile([128, C], F32, tag="cv", name=f"cv{qt}"))
                cand_p.append(cand_pool.tile([128, C], U32, tag="cp", name=f"cp{qt}"))

            # --- main loop: windows outer, q-tiles inner ---
            for w in range(n_windows):
                aug_t = aug_pool.tile([3, W], BF16, tag="aug")
                nc.sync.dma_start(aug_t[:], aug_d[w, :, :])
                t_ma = ma_pool.tile([128, D_CH, W], F16, tag="ma", name=f"ma{w}")
                t_s8 = s8_pool.tile([128, D_CH, W], FP8, tag="s8", name=f"s8{w}")
                t_ma8 = ma8_pool.tile([128, D_CH, W], FP8, tag="ma8", name=f"ma8{w}")
                for kc in range(D_CH):
                    nc.sync.dma_start(t_ma[:, kc, :], ma_d[w, kc, :, :])
                    nc.sync.dma_start(t_s8[:, kc, :], s8_d[w, kc, :, :])
                    nc.sync.dma_start(t_ma8[:, kc, :], ma8_d[w, kc, :, :])

                # -m^2*2^11 broadcast to all partitions, once per window
                ps_aug = psa_pool.tile([128, W], F32, tag="psa")
                for half in (0, 1):
                    hs = slice(half * 512, (half + 1) * 512)
                    nc.tensor.matmul(ps_aug[:, hs], ones3[:], aug_t[:, hs],
                                     start=True, stop=True)
                negb = negb_pool.tile([128, W], F32, tag="negb")
                nc.scalar.copy(negb[:], ps_aug[:])

                for qt in range(n_qtiles):
                    qs = slice(qt * 128, (qt + 1) * 128)
                    ps = ps_pool.tile([128, W], F32, tag="ps")
                    for half in (0, 1):
                        o = ps[:, half * 512:(half + 1) * 512]
                        hs = slice(half * 512, (half + 1) * 512)
                        n_mm = D_CH + 2 * (D_CH // 2)
                        j = 0
                        for kc in range(D_CH):
                            nc.tensor.matmul(
                                o, t_qa[:, kc, qs], t_ma[:, kc, hs],
                                start=(j == 0), stop=(j == n_mm - 1))
                            j += 1
                        for h in range(D_CH // 2):
                            nc.tensor.matmul(
                                o, t_r8[:, 2 * h:2 * h + 2, qs],
                                t_ma8[:, 2 * h:2 * h + 2, hs],
                                start=False, stop=(j == n_mm - 1),
                                perf_mode=DRMODE)
                            j += 1
                        for h in range(D_CH // 2):
                            nc.tensor.matmul(
                                o, t_qa8[:, 2 * h:2 * h + 2, qs],
                                t_s8[:, 2 * h:2 * h + 2, hs],
                                start=False, stop=(j == n_mm - 1),
                                perf_mode=DRMODE)
                            j += 1

                    # v' = ps + (-m^2*2^11), then top-8 per 1024-wide window
                    wnd = wnd_pool.tile([128, W], F32, tag="wnd")
                    nc.vector.tensor_tensor(
                        out=wnd[:], in0=ps[:], in1=negb[:], op=OP.add)
                    cv = cand_v[qt]
                    cp = cand_p[qt]
                    s0 = 8 * w
                    nc.vector.max(cv[:, s0:s0 + 8], wnd[:])
                    nc.vector.max_index(cp[:, s0:s0 + 8], cv[:, s0:s0 + 8], wnd[:])

            # --- merge per q-tile ---
            BIGU = 1 << 30
            for qt in range(n_qtiles):
                cv = cand_v[qt]
                cp = cand_p[qt]
                m16 = small_pool.tile([128, 16], F32, tag="m16")
                pos = small_pool.tile([128, 16], U32, tag="pos")
                cv_scr = merge_pool.tile([128, C], F32, tag="cvscr")
                nc.vector.max(m16[:, 0:8], cv[:])
                nc.vector.max_index(pos[:, 0:8], m16[:, 0:8], cv[:])
                nc.vector.match_replace(cv_scr[:], m16[:, 0:8], cv[:], NEG_BIG)
                nc.vector.max(m16[:, 8:16], cv_scr[:])
                nc.vector.max_index(pos[:, 8:16], m16[:, 8:16], cv_scr[:])

                # window base = (slot >> 3) << 10, since 8 cands per window
                wbase = small_pool.tile([128, 16], U32, tag="wbase")
                nc.vector.tensor_scalar(
                    wbase[:], pos[:], 3, 10,
                    op0=OP.logical_shift_right, op1=OP.logical_shift_left)
                posf = small_pool.tile([128, 16], F32, tag="posf")
                nc.vector.tensor_copy(posf[:], pos[:])
                cpf = merge_pool.tile([128, C], F32, tag="cpf")
                nc.vector.tensor_copy(cpf[:], cp[:])

                l9 = small_pool.tile([128, K], F32, tag="l9")
                for j in range(K):
                    # (iota == pos_j) * cp has one nonzero term -> fused
                    # accum_out sum extracts cp[pos_j] in a single pass
                    msk = merge_pool.tile([128, C], F32, tag="msk")
                    nc.vector.scalar_tensor_tensor(
                        msk[:], iota_f[:], posf[:, j:j + 1], cpf[:],
                        op0=OP.is_equal, op1=OP.mult,
                        accum_out=l9[:, j:j + 1])
                l9u = small_pool.tile([128, K], U32, tag="l9u")
                nc.vector.tensor_copy(l9u[:], l9[:])
                g9 = small_pool.tile([128, K], U32, tag="g9")
                nc.vector.tensor_tensor(
                    out=g9[:], in0=l9u[:], in1=wbase[:, 0:K], op=OP.add)

                v9 = small_pool.tile([128, K], F32, tag="v9")
                nc.vector.tensor_copy(v9[:, 0:8], m16[:, 0:8])
                nc.vector.tensor_copy(v9[:, 8:9], m16[:, 8:9])
                q2t = small_pool.tile([128, 1], F32, tag="q2t")
                nc.sync.dma_start(q2t[:], q2s_d[qt * 128:(qt + 1) * 128, :])
                # d2 = (v' - q2*2^11) * -2^-11 ; clamp >= 0 ; dist = sqrt
                nc.vector.tensor_scalar(
                    v9[:], v9[:], q2t[:], -1.0 / SCALE,
                    op0=OP.subtract, op1=OP.mult)
                nc.vector.tensor_scalar(v9[:], v9[:], 0.0, None, op0=OP.max)
                d9 = small_pool.tile([128, K], F32, tag="d9")
                nc.scalar.activation(d9[:], v9[:], AF.Sqrt)
                i9 = small_pool.tile([128, K], I32, tag="i9")
                nc.vector.tensor_copy(i9[:], g9[:])
                nc.sync.dma_start(outd_d[qt * 128:(qt + 1) * 128, :], d9[:])
                nc.sync.dma_start(outi_d[qt * 128:(qt + 1) * 128, :], i9[:])

    nc.compile()
    return nc


def _prep_shared(memory: np.ndarray):
    """Memory-bank layout prep (identical for every core)."""
    M = memory.shape[0]
    MP = N_WINDOWS * W
    MT = np.zeros((D, MP), np.float32)
    MT[:, :M] = memory.T.astype(np.float32)

    B = MT * np.float32(2.0 ** 6)
    MA = B.astype(F16NP)
    S = B - MA.astype(np.float32)
    S8 = (S * np.float32(2.0 ** 5)).astype(FP8NP)
    MA8 = MT.astype(FP8NP)

    negm2 = -(memory.astype(np.float64) ** 2).sum(1) * SCALE
    a1 = negm2.astype(BF16NP)
    r1 = negm2 - a1.astype(np.float64)
    a2 = r1.astype(BF16NP)
    a3 = (r1 - a2.astype(np.float64)).astype(BF16NP)
    aug = np.zeros((3, MP), BF16NP)
    aug[0, :M] = a1
    aug[1, :M] = a2
    aug[2, :M] = a3
    aug[0, M:] = NEG_BIG

    # window-major layout: each (window, chunk) slab is one contiguous
    # block in DRAM (strided 2KB reads run ~8GB/s; contiguous ~200+GB/s)
    def wm(x):
        return np.ascontiguousarray(
            x.reshape(D_CH, 128, N_WINDOWS, W).transpose(2, 0, 1, 3))

    return {
        "ma": wm(MA),
        "s8": wm(S8),
        "ma8": wm(MA8),
        "aug": np.ascontiguousarray(
            aug.reshape(3, N_WINDOWS, W).transpose(1, 0, 2)),
    }


def _prep_core(q_core: np.ndarray, shared: dict):
    NQ = q_core.shape[0]
    q2s = ((q_core.astype(np.float64) ** 2).sum(1) * SCALE).astype(
        np.float32)[:, None]
    QT2 = np.ascontiguousarray((2.0 * q_core.astype(np.float64)).T.astype(np.float32))
    A = QT2 * np.float32(2.0 ** 5)
    QA = A.astype(F16NP)
    R = A - QA.astype(np.float32)
    R8 = (R * np.float32(2.0 ** 6)).astype(FP8NP)
    QA8 = QT2.astype(FP8NP)
    return {
        "qa": np.ascontiguousarray(QA.reshape(D_CH, 128, NQ)),
        "r8": np.ascontiguousarray(R8.reshape(D_CH, 128, NQ)),
        "qa8": np.ascontiguousarray(QA8.reshape(D_CH, 128, NQ)),
        "q2s": q2s,
        **shared,
    }


_NC_CACHE = {}


def _get_nc():
    key = (N_QTILES, N_WINDOWS)
    if key not in _NC_CACHE:
        _NC_CACHE[key] = _build_knn_nc(*key)
    return _NC_CACHE[key]


def kernel(query, memory, k, **run_kwargs):
    query = np.asarray(query, dtype=np.float32)
    memory = np.asarray(memory, dtype=np.float32)
    k = int(k)
    assert k == K, f"kernel hardcodes k={K}, got {k}"
    assert query.shape == (N_CORES * N_QTILES * 128, D), query.shape
    assert memory.shape[0] <= N_WINDOWS * W and memory.shape[1] == D

    nc = _get_nc()
    shared = _prep_shared(memory)
    nq_per = N_QTILES * 128
    in_maps = [
        _prep_core(query[c * nq_per:(c + 1) * nq_per], shared)
        for c in range(N_CORES)
    ]
    res = run_bass_kernel_spmd(nc, in_maps, list(range(N_CORES)), **run_kwargs)
    dist = np.concatenate([r["out_d"] for r in res.results], axis=0)
    idx = np.concatenate([r["out_i"] for r in res.results], axis=0)
    if run_kwargs:
        kernel.last_results = res
    return dist, idx.astype(np.int32)
